# revision 2
# baseline (speedup 1.0000x reference)
"""Trainium2 Bass kernel for quantized linear: out = (x @ w.T + bias) * scale.

Shapes (hardcoded): x[16384,1024] i32 (int8-range), w[4096,1024] i32 (int8-range),
scale[4096] f32, bias[4096] i32  ->  out[16384,4096] f32.

Strategy:
- Shard M (rows of x) across 8 cores: each core computes out[c*2048:(c+1)*2048, :].
  (Less DMA than the column-parallel hint: x is the big tensor and is NOT
  replicated; w (8MB bf16) is replicated instead.)
- int8-range values are exact in bf16, and every partial sum of the i32 matmul
  is an integer of magnitude <= 1024*128*128 = 2^24, exactly representable in
  fp32. So a bf16 matmul with fp32 PSUM accumulation is bit-exact.
- Compute out.T per core (lhsT = w.T tile, rhs = x.T tile) so the per-out-channel
  scale/bias land on PSUM partitions: dequant is ONE ScalarE activation
  (Identity: out = psum*scale + bias*scale, per-partition affine) per tile.
- Host does layout prep only (dtype cast + transpose/tiling); all FLOPs on device.

Schedule (v2 — startup + tail rework; steady state was already gap-free):
- Startup: the PE roofline is 218.45us/core; everything else is head/tail loss.
  The first two output tiles (nt=0,1) run "pair-k-outer": all 8 PSUM banks
  accumulate in parallel, one k-slice of x at a time, so the PE consumes x at
  1.7us per 512KB k-slice (warm) while DMA delivers one per ~1.4us. Weight
  slices for nt=0,1 are loaded as per-k 32KB tiles and the first x k-slice as
  4x128KB chunks, dispatched in first-use order across BOTH HWDGE rings
  (sync + scalar) so the first matmul issues at ~1.3us instead of ~7.8us.
- Steady state (nt=3..30): k-outer per nt, 4 PSUM banks, dequant alternates
  ScalarE/VectorE (halves eviction latency), ONE batched 1MB store per nt on
  the sync ring (the per-chunk stores on ScalarE cost ~600ns of sequencer
  time each and made ScalarE the tail bottleneck).
- nt=2 runs chunk-outer so its PSUM demand (1 bank/1.7us) matches the rate at
  which the pair phase's 8 banks drain.
- Tail (nt=31): chunk-outer with narrowing chunks (512,512,512,256,128,128);
  each bank's dequant+store starts as soon as its own 8-matmul chain stops,
  stores spread across both HWDGE rings. nt=30 stores in 2 halves so its
  completion isn't the kernel's last event.
"""

import os

import numpy as np
import ml_dtypes

M, K, N = 16384, 1024, 4096
NCORES = 8
MS = M // NCORES  # 2048 rows of x per core
P = 128
KO = K // P  # 8 k-tiles
NT = N // P  # 32 n-tiles (PSUM partition dim = out-channel)
MC = 512  # psum free dim (one bank of fp32)
NMC = MS // MC  # 4 m-chunks per core

_CACHE = {}
LAST_RESULTS = None  # stash of BassKernelResults for test harnesses


def _build():
    import concourse.mybir as mybir
    import concourse.tile as tile
    from concourse import bacc

    dt = mybir.dt
    nc = bacc.Bacc("TRN2", target_bir_lowering=False, debug=False, num_devices=NCORES)

    # Host-pretiled layouts (see kernel() below):
    #   xT[p, ko, m]      = x_shard[m, ko*128+p]          (bf16)
    #   wt[nt, p, ko, nl] = w[nt*128+nl, ko*128+p]        (bf16)
    #   sc[p, nt]         = scale[nt*128+p]               (f32)
    #   bi[p, nt]         = scale[nt*128+p]*bias[nt*128+p](f32)
    #   outT[n, m]        = out_shard[m, n]               (f32)
    xT = nc.dram_tensor("xT", [P, KO, MS], dt.bfloat16, kind="ExternalInput").ap()
    wt = nc.dram_tensor("wt", [NT, P, KO, P], dt.bfloat16, kind="ExternalInput").ap()
    sc = nc.dram_tensor("sc", [P, NT], dt.float32, kind="ExternalInput").ap()
    bi = nc.dram_tensor("bi", [P, NT], dt.float32, kind="ExternalInput").ap()
    outT = nc.dram_tensor("outT", [N, MS], dt.float32, kind="ExternalOutput").ap()
    outT_t = outT.rearrange("(nt p) m -> nt p m", p=P)

    with tile.TileContext(nc) as tc:
        with (
            tc.tile_pool(name="xpool", bufs=1) as xpool,
            tc.tile_pool(name="wkpool", bufs=16) as wkpool,
            tc.tile_pool(name="wpool", bufs=5) as wpool,
            tc.tile_pool(name="cpool", bufs=1) as cpool,
            tc.tile_pool(name="opool", bufs=3) as opool,
            tc.tile_pool(name="ofpool", bufs=8) as ofpool,
            tc.tile_pool(name="psum", bufs=8, space="PSUM") as psum_pool,
        ):
            # ---- dequant helper: ScalarE activation or VectorE tensor_scalar,
            # both compute ot = psum*scale[n] + bias[n]*scale[n] per partition.
            sc_sb = None
            bi_sb = None

            def dequant(eng, ot, ps, nt):
                if eng == "s":
                    nc.scalar.activation(
                        ot,
                        ps,
                        mybir.ActivationFunctionType.Identity,
                        bias=bi_sb[:, nt : nt + 1],
                        scale=sc_sb[:, nt : nt + 1],
                    )
                else:
                    nc.vector.tensor_scalar(
                        ot,
                        ps,
                        sc_sb[:, nt : nt + 1],
                        bi_sb[:, nt : nt + 1],
                        mybir.AluOpType.mult,
                        mybir.AluOpType.add,
                    )

            # ---- startup DMA program. Each engine's dma_starts execute in
            # program order on its own HWDGE ring (~600ns of sequencer time
            # each); the two rings (sync=SP, scalar=ACT) transfer in parallel.
            # Order within each ring is first-use order of the pair phase.
            wk = {}  # (nt, k) -> [P, P] weight slice for nt in {0, 1}

            def load_wk(eng, nt, k):
                t = wkpool.tile([P, P], dt.bfloat16, tag="wk", name=f"wk_{nt}_{k}")
                eng.dma_start(t[:], wt[nt, :, k, :])
                wk[(nt, k)] = t

            x0c = []  # first k-slice of x, split in 4 chunks to gate MM #1 less

            def load_x0c(eng, ci):
                t = xpool.tile([P, MC], dt.bfloat16, tag=f"x0c{ci}", name=f"x0c_{ci}")
                eng.dma_start(t[:], xT[:, 0, ci * MC : (ci + 1) * MC])
                x0c.append(t)

            x_ko = {}  # k -> [P, MS] x k-slice (k >= 1)

            def load_x(eng, ko):
                t = xpool.tile([P, MS], dt.bfloat16, tag=f"x{ko}", name=f"x_{ko}")
                eng.dma_start(t[:], xT[:, ko])
                x_ko[ko] = t

            w_tiles = {}  # nt -> [P, KO, P] full weight tile (nt >= 2)

            def load_w(eng, nt):
                t = wpool.tile([P, KO, P], dt.bfloat16, tag="w", name=f"w_{nt}")
                eng.dma_start(t[:], wt[nt])
                w_tiles[nt] = t

            # sync ring: first-MM weight, then x1 (needed ~4.6us in), then the
            # per-k weight slices interleaved with x2/x3.
            load_wk(nc.sync, 0, 0)
            load_x(nc.sync, 1)
            load_wk(nc.sync, 1, 0)
            load_wk(nc.sync, 0, 1)
            load_wk(nc.sync, 1, 1)
            load_x(nc.sync, 2)
            load_wk(nc.sync, 0, 2)
            load_wk(nc.sync, 1, 2)
            load_x(nc.sync, 3)
            for k in range(3, KO):
                load_wk(nc.sync, 0, k)
                load_wk(nc.sync, 1, k)
            # scalar ring: the 4 chunks of x k-slice 0 (chunk 0 gates MM #1),
            # then the back half of the x stream.
            for ci in range(NMC):
                load_x0c(nc.scalar, ci)
            for k in range(4, KO):
                load_x(nc.scalar, k)
            # gpsimd (SWDGE, ~1us latency — fine for far-future deps):
            # dequant constants and the first two steady-state weight tiles.
            sc_sb = cpool.tile([P, NT], dt.float32)
            nc.gpsimd.dma_start(sc_sb[:], sc)
            bi_sb = cpool.tile([P, NT], dt.float32)
            nc.gpsimd.dma_start(bi_sb[:], bi)
            load_w(nc.gpsimd, 2)
            load_w(nc.gpsimd, 3)

            # ---- warm-up: PE clock is HAM-throttled to 1.2 GHz until ~3.4us
            # of sustained activity. 3 dummy matmuls (no DMA dependency) keep
            # the PE busy 0.5-1.8us while the first real operands land, so the
            # HAM flips as early as possible.
            warm = cpool.tile([P, MC], dt.bfloat16)
            nc.vector.memset(warm[:], 0.0)
            warm_ps = psum_pool.tile([P, MC], dt.float32, tag="ps", name="warm_ps")
            for _ in range(3):
                nc.tensor.matmul(
                    warm_ps[:], lhsT=warm[:, :P], rhs=warm[:], start=True, stop=True
                )

            # ---- pair phase: nt=0 and nt=1 together, k-outer across all 8
            # PSUM banks. 8 MMs per x k-slice = 1.7us (warm) per 512KB of x,
            # which the DMA stream sustains — the PE never waits on x.
            pair_ps = {}
            for nt in (0, 1):
                for ci in range(NMC):
                    pair_ps[(nt, ci)] = psum_pool.tile(
                        [P, MC], dt.float32, tag="ps", name=f"ps_p{nt}_{ci}"
                    )
            for k in range(KO):
                for nt in (0, 1):
                    for ci in range(NMC):
                        off = ci * MC
                        rhs = x0c[ci][:] if k == 0 else x_ko[k][:, off : off + MC]
                        nc.tensor.matmul(
                            pair_ps[(nt, ci)][:],
                            lhsT=wk[(nt, k)][:],
                            rhs=rhs,
                            start=(k == 0),
                            stop=(k == KO - 1),
                        )
            obig = {}
            for nt in (0, 1):
                obig[nt] = opool.tile([P, MS], dt.float32, tag="o", name=f"o_{nt}")
                for ci in range(NMC):
                    dequant(
                        "s" if ci % 2 == 0 else "v",
                        obig[nt][:, ci * MC : (ci + 1) * MC],
                        pair_ps[(nt, ci)][:],
                        nt,
                    )
            nc.sync.dma_start(outT_t[0], obig[0][:])
            nc.scalar.dma_start(outT_t[1], obig[1][:])

            def x_rhs(k, off, wd):
                if k == 0:
                    ci = off // MC
                    return x0c[ci][:, off - ci * MC : off - ci * MC + wd]
                return x_ko[k][:, off : off + wd]

            # ---- nt=2: chunk-outer (each 8-MM chain needs one freed bank per
            # 1.7us — matches the drain rate of the pair phase's 8 banks).
            w_sb = w_tiles.pop(2)
            load_w(nc.sync, 4)
            o2 = opool.tile([P, MS], dt.float32, tag="o", name="o_2")
            for ci in range(NMC):
                ps = psum_pool.tile([P, MC], dt.float32, tag="ps", name=f"ps_2_{ci}")
                for k in range(KO):
                    nc.tensor.matmul(
                        ps[:],
                        lhsT=w_sb[:, k],
                        rhs=x_rhs(k, ci * MC, MC),
                        start=(k == 0),
                        stop=(k == KO - 1),
                    )
                dequant(
                    "s" if ci % 2 == 0 else "v",
                    o2[:, ci * MC : (ci + 1) * MC],
                    ps[:],
                    2,
                )
            nc.sync.dma_start(outT_t[2], o2[:])

            # ---- steady state: nt=3..30, k-outer, one batched store per nt.
            for nt in range(3, NT - 1):
                if nt + 2 < NT:
                    load_w(nc.sync, nt + 2)
                w_sb = w_tiles.pop(nt)
                psums = [
                    psum_pool.tile([P, MC], dt.float32, tag="ps", name=f"ps_{nt}_{ci}")
                    for ci in range(NMC)
                ]
                for k in range(KO):
                    for ci in range(NMC):
                        nc.tensor.matmul(
                            psums[ci][:],
                            lhsT=w_sb[:, k],
                            rhs=x_rhs(k, ci * MC, MC),
                            start=(k == 0),
                            stop=(k == KO - 1),
                        )
                ot = opool.tile([P, MS], dt.float32, tag="o", name=f"o_{nt}")
                for ci in range(NMC):
                    dequant(
                        "s" if ci % 2 == 0 else "v",
                        ot[:, ci * MC : (ci + 1) * MC],
                        psums[ci][:],
                        nt,
                    )
                if nt < NT - 2:
                    nc.sync.dma_start(outT_t[nt], ot[:])
                else:
                    # nt=30: store in halves so its 1MB doesn't become the
                    # kernel's last DMA completion.
                    nc.sync.dma_start(outT_t[nt, :, :1024], ot[:, :1024])
                    nc.sync.dma_start(outT_t[nt, :, 1024:], ot[:, 1024:])

            # ---- tail: nt=31 chunk-outer with narrowing chunks; each bank
            # dequants+stores as soon as its own chain stops. Stores spread
            # across both HWDGE rings.
            nt = NT - 1
            w_sb = w_tiles.pop(nt)
            chunks = [
                (0, 512),
                (512, 512),
                (1024, 512),
                (1536, 256),
                (1792, 128),
                (1920, 128),
            ]
            for ci, (off, wd) in enumerate(chunks):
                ps = psum_pool.tile([P, wd], dt.float32, tag="ps", name=f"ps_t_{ci}")
                for k in range(KO):
                    nc.tensor.matmul(
                        ps[:],
                        lhsT=w_sb[:, k],
                        rhs=x_rhs(k, off, wd),
                        start=(k == 0),
                        stop=(k == KO - 1),
                    )
                ot = ofpool.tile([P, MC], dt.float32, tag="of", name=f"of_{ci}")
                ot = ot[:, :wd]
                dequant("s" if ci % 2 == 0 else "v", ot, ps[:], nt)
                if ci % 2 == 1:
                    nc.sync.dma_start(outT_t[nt, :, off : off + wd], ot)
                else:
                    nc.scalar.dma_start(outT_t[nt, :, off : off + wd], ot)

    nc.compile()
    return nc


def _get_nc():
    if "nc" not in _CACHE:
        _CACHE["nc"] = _build()
    return _CACHE["nc"]


def _try_install_ntff_hook():
    """Best-effort: register the axon NTFF profiling hook (the agent image's
    antenv lacks axon_hooks). Returns True if tracing is usable."""
    try:
        import sys
        import types

        import antenv

        if "antenv.axon_hooks" not in sys.modules:
            mod = types.ModuleType("antenv.axon_hooks")
            state = {"hook": None}
            mod.set_axon_ntff_profile_hook = lambda h: state.__setitem__("hook", h)
            mod.get_axon_ntff_profile_hook = lambda: state["hook"]
            sys.modules["antenv.axon_hooks"] = mod
            antenv.axon_hooks = mod

            from trn_agent_boot.trn_boot import _ntff_profile_via_ctypes

            hook = _ntff_profile_via_ctypes("/opt/axon/libaxon_pjrt.so")
            if hook is not None:
                mod.set_axon_ntff_profile_hook(hook)
        return True
    except Exception:
        return False


def kernel(**inputs) -> np.ndarray:
    global LAST_RESULTS
    from concourse.bass_utils import run_bass_kernel_spmd

    x = np.asarray(inputs["x"])
    w = np.asarray(inputs["weight"])
    scale = np.asarray(inputs["scale"], dtype=np.float32)
    bias = np.asarray(inputs["bias"])

    bf16 = ml_dtypes.bfloat16
    nc = _get_nc()

    # weight -> [nt, k_local(part), ko, n_local]
    wt = np.ascontiguousarray(
        w.astype(bf16).reshape(NT, P, KO, P).transpose(0, 3, 2, 1)
    )
    sc = np.ascontiguousarray(scale.reshape(NT, P).T)
    bi = np.ascontiguousarray((bias.astype(np.float32) * scale).reshape(NT, P).T)

    in_maps = []
    for c in range(NCORES):
        xs = x[c * MS : (c + 1) * MS].astype(bf16)  # [MS, K]
        xt = np.ascontiguousarray(xs.T.reshape(KO, P, MS).transpose(1, 0, 2))
        in_maps.append({"xT": xt, "wt": wt, "sc": sc, "bi": bi})

    trace = os.environ.get("BASS_TRACE", "0") == "1" and _try_install_ntff_hook()
    try:
        LAST_RESULTS = run_bass_kernel_spmd(
            nc, in_maps, core_ids=list(range(NCORES)), trace=trace
        )
    except Exception:
        if not trace:
            raise
        # Tracing plumbing is environment-dependent; never let it take down
        # the actual computation.
        os.environ["BASS_NEVER_TRACE"] = "1"
        LAST_RESULTS = run_bass_kernel_spmd(
            nc, in_maps, core_ids=list(range(NCORES)), trace=False
        )

    out = np.empty((M, N), dtype=np.float32)
    for c in range(NCORES):
        out[c * MS : (c + 1) * MS] = LAST_RESULTS.results[c]["outT"].T
    return out
